# revision 48
# baseline (speedup 1.0000x reference)
"""Chf (characteristic-function) loss kernel for Trainium2, SPMD over 8 cores.

Math: the reference builds cos/sin templates over a (u,v) frequency grid and
an N = W*H pixel grid with angle[u,v,(w,h)] = freq[v]*x[w] + freq[u]*y[h],
then contracts against the flattened image. Because the angle is separable,
cos/sin addition formulas factor the contraction into per-axis pieces:

  chf_real[b,u,v] = sum_{h,w} (Cx[v,w]*Cy[u,h] - Sx[v,w]*Sy[u,h]) * D[b,h,w]
  chf_img [b,u,v] = sum_{h,w} (Sx[v,w]*Cy[u,h] + Cx[v,w]*Sy[u,h]) * D[b,h,w]

with Cx[v,w] = cos(freq[v]*x[w]) etc. So per batch it is two 128x128x128
GEMM stages instead of a (4096 x 16384) template GEMM plus 134M cos/sin
evaluations. Sharding: data-parallel, 2 batches per core; each core emits
per-batch sum-of-squares of (derived - chf); host does sqrt/scale/mean.
"""

import os
import sys

import numpy as np

for _p in ("/opt/trn_rl_repo", "/root/.axon_site/_ro/trn_rl_repo"):
    if os.path.isdir(_p) and _p not in sys.path:
        sys.path.insert(0, _p)

from concourse import bacc, bass, mybir, tile  # noqa: E402
from concourse.bass_utils import run_bass_kernel_spmd  # noqa: E402

CHF_STEP = 32
CHF_TIK = 0.05
SAMPLE_STEP = 1.0
B, H, W = 16, 128, 128
S2 = 2 * CHF_STEP  # 64
N_CORES = 8
BPC = B // N_CORES  # batches per core
F32 = mybir.dt.float32


def _trig_constants():
    # x_axis == y_axis and the u/v freq grids are identical (H == W), so the
    # per-axis cos/sin factor matrices coincide: CxT == CyT == C, SxT == SyT == S.
    x = SAMPLE_STEP / 2 + SAMPLE_STEP * np.arange(W, dtype=np.float64)
    freq = np.arange(-CHF_STEP, CHF_STEP, dtype=np.float64) * CHF_TIK
    ang = x[:, None] * freq[None, :]  # (W, S2)
    c, s = np.cos(ang), np.sin(ang)
    # slab [-S | C | S | 1 | 0-pad]: cols S2:3*S2 = [C|S] (stage-1 rhs, and
    # stage-2 rhs for the P1_c half); cols 0:2*S2 = [-S|C] (stage-2 rhs for
    # the P1_s half); col 3*S2 = ones for the cross-partition reduce matmul.
    # Zero-padding widens every matmul rhs to 256 moving columns, which puts
    # float32r matmuls on the PE's 1-cycle/row fast path; the junk products
    # land in ignored PSUM columns.
    ones = np.ones((W, 1), dtype=np.float64)
    pad = np.zeros((W, 2 * S2 - 1), dtype=np.float64)
    trig = np.ascontiguousarray(
        np.concatenate([-s, c, s, ones, pad], axis=1).astype(np.float32)
    )  # (128, S2 + 256 = 320)
    return trig


def _build_nc():
    # Bass.__init__ emits four const-AP memsets plus a full all-engine
    # barrier (with per-engine DRAINs) ahead of the kernel body. This kernel
    # reads none of the const APs (the squared-reduce runs on DVE with an
    # immediate scalar), so skip the memsets and use the semaphore-only
    # barrier: both otherwise land inside the measured window before the
    # first DMA. Patches are scoped to __init__ only — Tile's tail
    # drain+barrier and any later memsets are unaffected.
    _orig_barrier = bass.Bass.all_engine_barrier
    _orig_memset = bass.BassGpSimd.memset

    def _sem_only_barrier(self, *, sem_only=False):
        _orig_barrier(self, sem_only=True)

    bass.Bass.all_engine_barrier = _sem_only_barrier
    bass.BassGpSimd.memset = lambda self, ap, constant: None
    try:
        nc = bacc.Bacc("TRN2", target_bir_lowering=False, debug=False)
    finally:
        bass.Bass.all_engine_barrier = _orig_barrier
        bass.BassGpSimd.memset = _orig_memset
    F32R = mybir.dt.float32r
    dnn = nc.dram_tensor("dnn", [BPC, H, W], F32R, kind="ExternalInput")
    chf = nc.dram_tensor("chf", [BPC, S2, S2, 2], F32, kind="ExternalInput")
    trig = nc.dram_tensor("trig", [H, S2 + 256], F32R, kind="ExternalInput")
    ssq = nc.dram_tensor("ssq", [1, BPC], F32, kind="ExternalOutput")

    with tile.TileContext(nc) as tc:
        with (
            tc.tile_pool(name="const", bufs=1) as cpool,
            tc.tile_pool(name="work", bufs=2) as wpool,
            tc.tile_pool(name="psum", bufs=2, space="PSUM") as ppool,
        ):
            # DMAs ordered by first use, split across the two HWDGE queues:
            #   sync:   [C|S] slab, then [-S], then chf
            #   scalar: dnn batch 0, then dnn batch 1
            tg = cpool.tile([H, S2 + 256], F32R)
            nc.sync.dma_start(tg[:, S2:], trig[:, S2:])  # [C|S|1|pad]: stage 1
            d_all = cpool.tile([H, BPC, W], F32R)
            nc.scalar.dma_start(d_all[:, 0, :], dnn[0])
            nc.scalar.dma_start(d_all[:, 1, :], dnn[1])
            nc.sync.dma_start(tg[:, 0:S2], trig[:, 0:S2])  # [-S]: stage 2 only
            cht = cpool.tile([S2, BPC, S2, 2], F32)
            nc.sync.dma_start(cht[:], chf.rearrange("b u v c -> u b v c"))
            cols = cpool.tile([S2, BPC], F32)

            diffs = []
            for b in range(BPC):
                # stage 1 (N=256 f32r): p1 cols S2:3*S2 = [D.T@C | D.T@S]
                p1 = ppool.tile([W, 256], F32, tag="p1")
                nc.tensor.matmul(
                    p1[:], d_all[:, b, :], tg[:, S2 : S2 + 256],
                    start=True, stop=True,
                )
                # split the PSUM->SBUF copy (with fp32->fp32r rounding) so
                # stage-2's first matmul only waits on the P1_c half
                p1s = wpool.tile([W, 2 * S2], F32R, tag="p1s")
                nc.vector.tensor_copy(p1s[:, 0:S2], p1[:, 0:S2])
                nc.vector.tensor_copy(p1s[:, S2 : 2 * S2], p1[:, S2 : 2 * S2])
                # stage 2 (N=256 f32r, only out cols 0:2*S2 meaningful):
                #   p2[u, 0:2*S2] = [real|img] = P1_c.T@[C|S] + P1_s.T@[-S|C]
                p2 = ppool.tile([S2, 256], F32, tag="p2")
                nc.tensor.matmul(
                    p2[:], p1s[:, 0:S2], tg[:, S2 : S2 + 256],
                    start=True, stop=False,
                )
                nc.tensor.matmul(
                    p2[:], p1s[:, S2 : 2 * S2], tg[:, 0:256],
                    start=False, stop=True,
                )
                # diff[u, c, v] = p2[u, c*S2+v] - chf[b, u, v, c]
                diff = wpool.tile([S2, 2, S2], F32, tag=f"diff{b}")
                nc.vector.tensor_sub(
                    diff[:],
                    p2[:, 0 : 2 * S2].rearrange("u (c v) -> u c v", c=2),
                    cht[:, b, :, :].rearrange("u v c -> u c v"),
                )
                diffs.append(diff)
            # squared-reduce + cross-partition reduce + output, LAST batch
            # first: its chain is the critical path, so it gets DVE priority
            # and the faster-dispatching sync DMA queue; the earlier batch's
            # chain overlaps it on the scalar queue.
            for b in reversed(range(BPC)):
                # sq = diff * diff with fused row-sum into cols[:, b]
                sq = wpool.tile([S2, 2, S2], F32, tag="sq")
                nc.vector.scalar_tensor_tensor(
                    out=sq[:],
                    in0=diffs[b][:],
                    scalar=1.0,
                    in1=diffs[b][:],
                    op0=mybir.AluOpType.mult,
                    op1=mybir.AluOpType.mult,
                    accum_out=cols[:, b : b + 1],
                )
                pss = ppool.tile([1, 1], F32, tag="pss")
                nc.tensor.matmul(
                    pss[:],
                    tg[0:S2, 3 * S2 : 3 * S2 + 1].bitcast(F32),
                    cols[:, b : b + 1],
                    start=True, stop=True,
                )
                outt = wpool.tile([1, 1], F32, tag="outt")
                nc.vector.tensor_copy(outt[:], pss[:])
                eng = nc.sync if b == BPC - 1 else nc.scalar
                eng.dma_start(ssq[0:1, b : b + 1], outt[:])

    nc.compile()
    return nc


_NC_CACHE = None


def _get_nc():
    global _NC_CACHE
    if _NC_CACHE is None:
        _NC_CACHE = _build_nc()
    return _NC_CACHE


def kernel(dnn_output: np.ndarray, chf: np.ndarray) -> np.ndarray:
    dnn_output = np.ascontiguousarray(dnn_output, dtype=np.float32)
    chf = np.ascontiguousarray(chf, dtype=np.float32)
    trig = _trig_constants()
    in_maps = [
        {
            "dnn": dnn_output[c * BPC : (c + 1) * BPC],
            "chf": chf[c * BPC : (c + 1) * BPC],
            "trig": trig,
        }
        for c in range(N_CORES)
    ]
    nc = _get_nc()
    results = run_bass_kernel_spmd(nc, in_maps, list(range(N_CORES))).results
    ssq = np.concatenate([np.asarray(r["ssq"]).reshape(-1) for r in results])
    loss = np.sqrt(ssq.astype(np.float64)).sum() * CHF_TIK / B
    return np.float32(loss)


# revision 49
# speedup vs baseline: 1.0164x; 1.0164x over previous
"""Chf (characteristic-function) loss kernel for Trainium2, SPMD over 8 cores.

Math: the reference builds cos/sin templates over a (u,v) frequency grid and
an N = W*H pixel grid with angle[u,v,(w,h)] = freq[v]*x[w] + freq[u]*y[h],
then contracts against the flattened image. Because the angle is separable,
cos/sin addition formulas factor the contraction into per-axis pieces:

  chf_real[b,u,v] = sum_{h,w} (Cx[v,w]*Cy[u,h] - Sx[v,w]*Sy[u,h]) * D[b,h,w]
  chf_img [b,u,v] = sum_{h,w} (Sx[v,w]*Cy[u,h] + Cx[v,w]*Sy[u,h]) * D[b,h,w]

with Cx[v,w] = cos(freq[v]*x[w]) etc. So per batch it is two 128x128x128
GEMM stages instead of a (4096 x 16384) template GEMM plus 134M cos/sin
evaluations. Sharding: data-parallel, 2 batches per core; each core emits
per-batch sum-of-squares of (derived - chf); host does sqrt/scale/mean.
"""

import os
import sys

import numpy as np

for _p in ("/opt/trn_rl_repo", "/root/.axon_site/_ro/trn_rl_repo"):
    if os.path.isdir(_p) and _p not in sys.path:
        sys.path.insert(0, _p)

from concourse import bacc, bass, mybir, tile  # noqa: E402
from concourse.bass_utils import run_bass_kernel_spmd  # noqa: E402

CHF_STEP = 32
CHF_TIK = 0.05
SAMPLE_STEP = 1.0
B, H, W = 16, 128, 128
S2 = 2 * CHF_STEP  # 64
N_CORES = 8
BPC = B // N_CORES  # batches per core
F32 = mybir.dt.float32


def _trig_constants():
    # x_axis == y_axis and the u/v freq grids are identical (H == W), so the
    # per-axis cos/sin factor matrices coincide: CxT == CyT == C, SxT == SyT == S.
    x = SAMPLE_STEP / 2 + SAMPLE_STEP * np.arange(W, dtype=np.float64)
    freq = np.arange(-CHF_STEP, CHF_STEP, dtype=np.float64) * CHF_TIK
    ang = x[:, None] * freq[None, :]  # (W, S2)
    c, s = np.cos(ang), np.sin(ang)
    # slab [-S | C | S | 1 | 0-pad]: cols S2:3*S2 = [C|S] (stage-1 rhs, and
    # stage-2 rhs for the P1_c half); cols 0:2*S2 = [-S|C] (stage-2 rhs for
    # the P1_s half); col 3*S2 = ones for the cross-partition reduce matmul.
    # Zero-padding widens every matmul rhs to 256 moving columns, which puts
    # float32r matmuls on the PE's 1-cycle/row fast path; the junk products
    # land in ignored PSUM columns.
    ones = np.ones((W, 1), dtype=np.float64)
    pad = np.zeros((W, 2 * S2 - 1), dtype=np.float64)
    trig = np.ascontiguousarray(
        np.concatenate([-s, c, s, ones, pad], axis=1).astype(np.float32)
    )  # (128, S2 + 256 = 320)
    return trig


def _build_nc():
    # Bass.__init__ emits four const-AP memsets plus a full all-engine
    # barrier (with per-engine DRAINs) ahead of the kernel body. This kernel
    # reads none of the const APs (the squared-reduce runs on DVE with an
    # immediate scalar), so skip the memsets and use the semaphore-only
    # barrier: both otherwise land inside the measured window before the
    # first DMA. Patches are scoped to __init__ only — Tile's tail
    # drain+barrier and any later memsets are unaffected.
    _orig_barrier = bass.Bass.all_engine_barrier
    _orig_memset = bass.BassGpSimd.memset

    def _sem_only_barrier(self, *, sem_only=False):
        _orig_barrier(self, sem_only=True)

    bass.Bass.all_engine_barrier = _sem_only_barrier
    bass.BassGpSimd.memset = lambda self, ap, constant: None
    try:
        nc = bacc.Bacc("TRN2", target_bir_lowering=False, debug=False)
    finally:
        bass.Bass.all_engine_barrier = _orig_barrier
        bass.BassGpSimd.memset = _orig_memset
    F32R = mybir.dt.float32r
    dnn = nc.dram_tensor("dnn", [BPC, H, W], F32R, kind="ExternalInput")
    chf = nc.dram_tensor("chf", [BPC, S2, S2, 2], F32, kind="ExternalInput")
    trig = nc.dram_tensor("trig", [H, S2 + 256], F32R, kind="ExternalInput")
    ssq = nc.dram_tensor("ssq", [1, BPC], F32, kind="ExternalOutput")

    with tile.TileContext(nc) as tc:
        with (
            tc.tile_pool(name="const", bufs=1) as cpool,
            tc.tile_pool(name="work", bufs=2) as wpool,
            tc.tile_pool(name="psum", bufs=2, space="PSUM") as ppool,
        ):
            # DMAs ordered by first use, split across the two HWDGE queues:
            #   sync:   [C|S] slab, then [-S], then chf
            #   scalar: dnn batch 0, then dnn batch 1
            tg = cpool.tile([H, S2 + 256], F32R)
            nc.sync.dma_start(tg[:, S2:], trig[:, S2:])  # [C|S|1|pad]: stage 1
            d_all = cpool.tile([H, BPC, W], F32R)
            nc.scalar.dma_start(d_all[:, 0, :], dnn[0])
            nc.scalar.dma_start(d_all[:, 1, :], dnn[1])
            nc.sync.dma_start(tg[:, 0:S2], trig[:, 0:S2])  # [-S]: stage 2 only
            cht = cpool.tile([S2, BPC, S2, 2], F32)
            nc.sync.dma_start(cht[:], chf.rearrange("b u v c -> u b v c"))
            cols = cpool.tile([S2, BPC], F32)

            for b in range(BPC):
                # stage 1 (N=256 f32r): p1 cols S2:3*S2 = [D.T@C | D.T@S]
                p1 = ppool.tile([W, 256], F32, tag="p1")
                nc.tensor.matmul(
                    p1[:], d_all[:, b, :], tg[:, S2 : S2 + 256],
                    start=True, stop=True,
                )
                # split the PSUM->SBUF copy (with fp32->fp32r rounding) so
                # stage-2's first matmul only waits on the P1_c half
                p1s = wpool.tile([W, 2 * S2], F32R, tag="p1s")
                nc.vector.tensor_copy(p1s[:, 0:S2], p1[:, 0:S2])
                nc.vector.tensor_copy(p1s[:, S2 : 2 * S2], p1[:, S2 : 2 * S2])
                # stage 2 (N=256 f32r, only out cols 0:2*S2 meaningful):
                #   p2[u, 0:2*S2] = [real|img] = P1_c.T@[C|S] + P1_s.T@[-S|C]
                p2 = ppool.tile([S2, 256], F32, tag="p2")
                nc.tensor.matmul(
                    p2[:], p1s[:, 0:S2], tg[:, S2 : S2 + 256],
                    start=True, stop=False,
                )
                nc.tensor.matmul(
                    p2[:], p1s[:, S2 : 2 * S2], tg[:, 0:256],
                    start=False, stop=True,
                )
                # diff[u, c, v] = p2[u, c*S2+v] - chf[b, u, v, c]
                diff = wpool.tile([S2, 2, S2], F32, tag="diff")
                nc.vector.tensor_sub(
                    diff[:],
                    p2[:, 0 : 2 * S2].rearrange("u (c v) -> u c v", c=2),
                    cht[:, b, :, :].rearrange("u v c -> u c v"),
                )
                # sq = diff * diff with fused row-sum into cols[:, b]
                sq = wpool.tile([S2, 2, S2], F32, tag="sq")
                nc.vector.scalar_tensor_tensor(
                    out=sq[:],
                    in0=diff[:],
                    scalar=1.0,
                    in1=diff[:],
                    op0=mybir.AluOpType.mult,
                    op1=mybir.AluOpType.mult,
                    accum_out=cols[:, b : b + 1],
                )
            # per-batch cross-partition reduce + output, emitted after both
            # batches' SUB/STT so batch 0's copy is not scheduled between
            # batch 1's DVE pair. Batch 0's chain (scalar DMA queue) overlaps
            # batch 1's, and the two output-DMA completion latencies overlap.
            for b in range(BPC):
                pss = ppool.tile([1, 1], F32, tag="pss")
                nc.tensor.matmul(
                    pss[:],
                    tg[0:S2, 3 * S2 : 3 * S2 + 1].bitcast(F32),
                    cols[:, b : b + 1],
                    start=True, stop=True,
                )
                outt = wpool.tile([1, 1], F32, tag="outt")
                nc.vector.tensor_copy(outt[:], pss[:])
                eng = nc.sync if b % 2 else nc.scalar
                eng.dma_start(ssq[0:1, b : b + 1], outt[:])

    nc.compile()
    return nc


_NC_CACHE = None


def _get_nc():
    global _NC_CACHE
    if _NC_CACHE is None:
        _NC_CACHE = _build_nc()
    return _NC_CACHE


def kernel(dnn_output: np.ndarray, chf: np.ndarray) -> np.ndarray:
    dnn_output = np.ascontiguousarray(dnn_output, dtype=np.float32)
    chf = np.ascontiguousarray(chf, dtype=np.float32)
    trig = _trig_constants()
    in_maps = [
        {
            "dnn": dnn_output[c * BPC : (c + 1) * BPC],
            "chf": chf[c * BPC : (c + 1) * BPC],
            "trig": trig,
        }
        for c in range(N_CORES)
    ]
    nc = _get_nc()
    results = run_bass_kernel_spmd(nc, in_maps, list(range(N_CORES))).results
    ssq = np.concatenate([np.asarray(r["ssq"]).reshape(-1) for r in results])
    loss = np.sqrt(ssq.astype(np.float64)).sum() * CHF_TIK / B
    return np.float32(loss)
